# revision 5
# baseline (speedup 1.0000x reference)
"""Multi-head attention (B=4, N=2048, C=1024, H=16, D=64) on 8 TRN2 NeuronCores.

Sharding: core c owns (batch b = c//2, sequence half = c%2) -> 1024 query
tokens, all 16 heads.  Each core computes K/V for the full 2048-token
sequence of its batch locally (v1: no collectives), runs attention for its
query half, and the output projection for its rows.  Output is purely
row-sharded -> host gather is a concat.

Device-side layout tricks (all transposes are done on the host):
- xT_aug  [1025, 2048] bf16: channel-major x with a ones row appended;
  own-half tokens first so the query range is cols [0, 1024) on every core.
- wqkvT_aug [1025, 3072] bf16: w_qkv^T with the bias as row 1024 (folded
  into the matmul via the ones row of xT_aug).
- Scores are computed transposed (S^T[m, n]) so no transpose is ever
  needed: softmax denominators come from a ones-column appended to V inside
  the PV matmul, and the normalization is a per-head broadcast multiply.
- All matmuls in bf16 (f32 PSUM accumulate).
"""

import numpy as np
import ml_dtypes

import concourse.bass as bass
import concourse.mybir as mybir
import concourse.tile as tile
from concourse import bacc
from concourse.bass_utils import run_bass_kernel_spmd

B, N, C = 4, 2048, 1024
H, D = 16, 64
SCALE = D ** -0.5
NCORES = 8
NQ = N // 2          # query tokens per core
M = N                # key/value tokens per core
CT = [128] * 8 + [1]  # contraction tiles over augmented channel dim (1025)

BF16 = mybir.dt.bfloat16
F32 = mybir.dt.float32

_CACHE = {}
LAST_RESULTS = None


def _build():
    nc = bacc.Bacc(
        "TRN2",
        target_bir_lowering=False,
        debug=False,
        enable_asserts=False,
        num_devices=NCORES,
    )
    xT = nc.dram_tensor("xT", [1025, M], BF16, kind="ExternalInput")
    wqkvT = nc.dram_tensor("wqkvT", [1025, 3 * C], BF16, kind="ExternalInput")
    wprojT = nc.dram_tensor("wprojT", [C, C], BF16, kind="ExternalInput")
    bproj = nc.dram_tensor("bproj", [C, 1], F32, kind="ExternalInput")
    yT = nc.dram_tensor("yT", [C, NQ], F32, kind="ExternalOutput")

    with tile.TileContext(nc) as tc:
        with (
            tc.tile_pool(name="persist", bufs=1) as pp,
            tc.tile_pool(name="psum", bufs=1, space="PSUM") as psp,
        ):
            lp = tc.alloc_tile_pool(name="qkv_in", bufs=1)
            # ---- load inputs ----
            x_sb = []
            wq_sb = []
            for ct in range(9):
                p = CT[ct]
                t = lp.tile([p, M], BF16, tag=f"x{ct}", name=f"x{ct}")
                nc.sync.dma_start(t[:, :], xT[ct * 128 : ct * 128 + p, :])
                x_sb.append(t)
                t = lp.tile([p, 3 * C], BF16, tag=f"wq{ct}", name=f"wq{ct}")
                nc.sync.dma_start(t[:, :], wqkvT[ct * 128 : ct * 128 + p, :])
                wq_sb.append(t)
            wp_sb = []
            bp_sb = []
            for i in range(8):
                t = pp.tile([128, C], BF16, tag=f"wp{i}", name=f"wp{i}")
                nc.sync.dma_start(t[:, :], wprojT[i * 128 : (i + 1) * 128, :])
                wp_sb.append(t)
                t = pp.tile([128, 1], F32, tag=f"bp{i}", name=f"bp{i}")
                nc.sync.dma_start(t[:, :], bproj[i * 128 : (i + 1) * 128, :])
                bp_sb.append(t)

            QT_sb = [pp.tile([128, NQ], BF16, tag=f"qt{i}", name=f"qt{i}") for i in range(8)]
            KT_sb = [pp.tile([128, M], BF16, tag=f"kt{i}", name=f"kt{i}") for i in range(8)]
            V_sb = [pp.tile([128, H, D + 1], BF16, tag=f"v{mt}", name=f"v{mt}") for mt in range(16)]
            A_sb = [pp.tile([128, NQ], BF16, tag=f"a{i}", name=f"a{i}") for i in range(8)]

            # ---- QKV projections ----
            # Q^T / K^T tiles: out[o_tile, n] = sum_c wqkvT[c, o]^T. x^T[c, n]
            for kind, base, dst, ncols in (("q", 0, QT_sb, NQ), ("k", C, KT_sb, M)):
                for ot in range(8):
                    for nch in range(ncols // 512):
                        ps = psp.tile([128, 512], F32, tag="mm", bufs=3, name="psqk")
                        for ct in range(9):
                            nc.tensor.matmul(
                                ps[:, :],
                                wq_sb[ct][:, base + ot * 128 : base + (ot + 1) * 128],
                                x_sb[ct][:, nch * 512 : (nch + 1) * 512],
                                start=(ct == 0),
                                stop=(ct == 8),
                            )
                        nc.vector.tensor_copy(
                            dst[ot][:, nch * 512 : (nch + 1) * 512], ps[:, :]
                        )
            # V tiles: out[m_tile, o] = x^T[c, m]^T . wqkvT[c, 2048+o]
            for mt in range(16):
                nc.vector.memset(V_sb[mt][:, :, D : D + 1], 1.0)
                for vch in range(2):
                    ps = psp.tile([128, 8, 64], F32, tag="mm", bufs=3, name="psv")
                    for ct in range(9):
                        nc.tensor.matmul(
                            ps[:, :, :],
                            x_sb[ct][:, mt * 128 : (mt + 1) * 128],
                            wq_sb[ct][:, 2 * C + vch * 512 : 2 * C + (vch + 1) * 512],
                            start=(ct == 0),
                            stop=(ct == 8),
                        )
                    nc.vector.tensor_copy(
                        V_sb[mt][:, vch * 8 : (vch + 1) * 8, 0:D], ps[:, :, :]
                    )

            lp.release()
            wp = tc.alloc_tile_pool(name="attnwork", bufs=1)
            # ---- attention, head by head ----
            for h in range(H):
                i, poff = h // 2, (h % 2) * 64
                pv = [psp.tile([65, 512], F32, tag=f"acc{j}", bufs=2, name=f"pv{j}") for j in range(2)]
                for mt in range(16):
                    for nch in range(2):
                        sp = psp.tile([128, 512], F32, tag="mm", bufs=3, name="pss")
                        nc.tensor.matmul(
                            sp[:, :],
                            KT_sb[i][poff : poff + 64, mt * 128 : (mt + 1) * 128],
                            QT_sb[i][poff : poff + 64, nch * 512 : (nch + 1) * 512],
                            start=True,
                            stop=True,
                        )
                        p = wp.tile([128, 512], BF16, tag="p", bufs=4, name="p")
                        nc.scalar.activation(
                            p[:, :], sp[:, :],
                            mybir.ActivationFunctionType.Exp, scale=SCALE,
                        )
                        nc.tensor.matmul(
                            pv[nch][:, :],
                            V_sb[mt][:, h, :],
                            p[:, :],
                            start=(mt == 0),
                            stop=(mt == 15),
                            skip_group_check=True,
                        )
                for nch in range(2):
                    r = wp.tile([1, 512], F32, tag="r", bufs=2, name="r")
                    nc.vector.reciprocal(r[:, :], pv[nch][64:65, :])
                    rb = wp.tile([64, 512], F32, tag="rb", bufs=2, name="rb")
                    nc.gpsimd.partition_broadcast(rb[:, :], r[:, :])
                    nc.vector.tensor_mul(
                        A_sb[i][poff : poff + 64, nch * 512 : (nch + 1) * 512],
                        pv[nch][0:64, :],
                        rb[:, :],
                    )

            wp.release()
            wp = tc.alloc_tile_pool(name="projwork", bufs=1)
            # ---- output projection ----
            for ot in range(8):
                for nch in range(2):
                    ps = psp.tile([128, 512], F32, tag=f"acc{nch}", bufs=2, name="psp")
                    for dd in range(8):
                        nc.tensor.matmul(
                            ps[:, :],
                            wp_sb[dd][:, ot * 128 : (ot + 1) * 128],
                            A_sb[dd][:, nch * 512 : (nch + 1) * 512],
                            start=(dd == 0),
                            stop=(dd == 7),
                        )
                    y = wp.tile([128, 512], F32, tag="y", bufs=3, name="y")
                    nc.vector.tensor_scalar_add(y[:, :], ps[:, :], bp_sb[ot][:, :])
                    nc.sync.dma_start(
                        yT[ot * 128 : (ot + 1) * 128, nch * 512 : (nch + 1) * 512],
                        y[:, :],
                    )
            wp.release()

    nc.compile()
    return nc


def kernel(x, w_qkv, b_qkv, w_proj, b_proj):
    global LAST_RESULTS
    bf = ml_dtypes.bfloat16
    x = np.asarray(x, np.float32)
    w_qkv = np.asarray(w_qkv, np.float32)
    b_qkv = np.asarray(b_qkv, np.float32)
    w_proj = np.asarray(w_proj, np.float32)
    b_proj = np.asarray(b_proj, np.float32)

    wqkvT = np.ascontiguousarray(
        np.vstack([w_qkv.T, b_qkv[None, :]]).astype(bf)
    )  # [1025, 3072]
    wprojT = np.ascontiguousarray(w_proj.T.astype(bf))  # [1024, 1024]
    bproj = np.ascontiguousarray(b_proj[:, None].astype(np.float32))  # [1024, 1]

    in_maps = []
    for core in range(NCORES):
        b, half = core // 2, core % 2
        xb = x[b]  # [2048, 1024]
        xcat = np.concatenate(
            [xb[half * NQ : (half + 1) * NQ], xb[(1 - half) * NQ : (2 - half) * NQ]],
            axis=0,
        )
        xT = np.vstack([xcat.T, np.ones((1, M), np.float32)]).astype(bf)
        in_maps.append(
            {
                "xT": np.ascontiguousarray(xT),
                "wqkvT": wqkvT,
                "wprojT": wprojT,
                "bproj": bproj,
            }
        )

    if "nc" not in _CACHE:
        _CACHE["nc"] = _build()
    nc = _CACHE["nc"]

    res = run_bass_kernel_spmd(nc, in_maps, core_ids=list(range(NCORES)))
    LAST_RESULTS = res

    out = np.empty((B, N, C), np.float32)
    for core in range(NCORES):
        b, half = core // 2, core % 2
        out[b, half * NQ : (half + 1) * NQ, :] = res.results[core]["yT"].T
    return out


if __name__ == "__main__":
    rng = np.random.default_rng(0)
    s = C ** -0.5
    ins = {
        "x": rng.standard_normal((B, N, C), np.float32),
        "w_qkv": (rng.standard_normal((3 * C, C), np.float32) * s).astype(np.float32),
        "b_qkv": (rng.standard_normal(3 * C, np.float32) * 0.02).astype(np.float32),
        "w_proj": (rng.standard_normal((C, C), np.float32) * s).astype(np.float32),
        "b_proj": (rng.standard_normal(C, np.float32) * 0.02).astype(np.float32),
    }
    y = kernel(**ins)
    print("out", y.shape, y.dtype, float(np.abs(y).mean()))
